# revision 30
# baseline (speedup 1.0000x reference)
"""Trainium2 Bass kernel for BaseMessageModule (GNN message passing).

Strategy (v6 — host-gathered embedding stream + device one-hots at DVE 2x,
software-pipelined 2 pairs ahead):
- Shard ATOMS across the 8 cores (3750 each). Host routes each pair to the
  core owning its receiving atom idx_i and sorts pairs by receiving atom.
- Pairs are cut into variable-base tiles: each tile covers <= 16 consecutive
  atoms and exactly CPT*128 pair slots. Tile t's pairs accumulate into a
  PSUM slice addressed by t (static schedule, SPMD-safe).
- The neighbor-embedding gather E[idx_j] is done on the HOST (pure integer
  indexing, like the routing/sorting): rows are shipped pre-ordered per pair
  slot as one contiguous bf16 stream, consumed with one plain 512KB DMA per
  super at full HBM bandwidth (v1's per-row SWDGE descriptor generation was
  the 219us critical path).
- Key linearity: segment-sum first, then apply W once per atom (20x less
  matmul), bias as count[n] * b.
- One-hot coefficient planes built ON DEVICE from per-chunk slot indices:
  cmp2 = is_equal(iota, ii) and O~ = cmp2 * Cu4 (Cu4 = [f, f*u0, f*u1,
  f*u2] from f/r). Layout (chunk, atom, k) with k innermost and x2-
  duplicated indices keeps every DVE operand innermost-stride-1 (2x mode)
  AND keeps each chunk's 64 rhs columns contiguous (65ns/MM vs 232ns
  strided). The build for pair b+2 is emitted before pair b's tail so the
  DVE FIFO never blocks the next pair's matmuls.
- Per 128-pair chunk: PSUM[f, (a,k)] += E_chunk.T @ O~_chunk.
- Tail per 2 supers (16 tiles): acc2 drained first (U on scalar, radial on
  vector) to free PSUM, W-transform + count*b bias on PE, squares on
  scalar from PSUM, norm-adds on GPSIMD, sqrt on scalar. Output stays
  [f, slot]-major bf16; the HOST does the final transpose (pure layout).

All floating-point arithmetic runs on device. Host work is integer index
manipulation (routing/sorting/padding/gather = sharding) and array layout.
"""

from contextlib import ExitStack

import ml_dtypes
import numpy as np

import concourse.bass as bass
import concourse.bacc as bacc
import concourse.tile as tile
from concourse import mybir
from concourse.bass_utils import run_bass_kernel_spmd

F = 128
ATILE = 16  # atom window per tile
KBLK = 4  # coefficient planes: radial, u0, u1, u2
CHUNK = 128  # pairs per matmul chunk
CPT = 2  # chunks (of 128 pair slots) per tile
TPS = 8  # tiles per super
SUP_C = TPS * CPT  # chunks per super (16)
EW = SUP_C * F  # E cols per super (2048)
PF = 6  # E-stream prefetch depth (supers in flight)

TW = ATILE * KBLK  # one-hot cols per chunk / psum cols per tile (64)


def _ap(t_ap, free_dims, off=0):
    """Custom AP view over the same partitions as t_ap with given free dims."""
    return bass.AP(t_ap.tensor, t_ap.offset + off, [t_ap.ap[0]] + list(free_dims))


def build_nc(N, T, n_cores):
    """Build the SPMD program for one core with T pair tiles."""
    CH = T * CPT  # chunks per core
    n_super = T // TPS
    n_sp = T // (2 * TPS)  # super-pairs (tail granularity)
    BW2 = 2 * TPS * 3 * ATILE  # u-plane cols per super-pair (768)
    C2 = 2 * SUP_C  # chunks per super-pair (32)

    fp = mybir.dt.float32
    bf = mybir.dt.bfloat16

    nc = bacc.Bacc("TRN2", target_bir_lowering=False, debug=False,
                   num_devices=n_cores)

    egd = nc.dram_tensor("egd", [128, n_super * EW], bf, kind="ExternalInput")
    iid2 = nc.dram_tensor("iid2", [128, 2 * CH], bf, kind="ExternalInput")
    fT = nc.dram_tensor("fT", [128, CH], fp, kind="ExternalInput")
    r0T = nc.dram_tensor("r0T", [128, CH], bf, kind="ExternalInput")
    r1T = nc.dram_tensor("r1T", [128, CH], bf, kind="ExternalInput")
    r2T = nc.dram_tensor("r2T", [128, CH], bf, kind="ExternalInput")
    c3d = nc.dram_tensor("c3d", [1, n_sp * BW2], bf, kind="ExternalInput")
    wTd = nc.dram_tensor("wTd", [F, F], fp, kind="ExternalInput")
    browd = nc.dram_tensor("browd", [1, F], fp, kind="ExternalInput")
    outd = nc.dram_tensor("outd", [128, n_sp * 512], bf,
                          kind="ExternalOutput")

    mult, add = mybir.AluOpType.mult, mybir.AluOpType.add
    iseq = mybir.AluOpType.is_equal

    with tile.TileContext(nc) as tc, ExitStack() as ctx:
        cpool = ctx.enter_context(tc.tile_pool(name="const", bufs=1))
        mpool = ctx.enter_context(tc.tile_pool(name="main", bufs=1))

        with tc.tile_pool(name="esup", bufs=PF // 2 + 1) as epool, \
             tc.tile_pool(name="cmp", bufs=3) as cmpool, \
             tc.tile_pool(name="ohp", bufs=3) as opool, \
             tc.tile_pool(name="pacc", bufs=3, space="PSUM") as ppool, \
             tc.tile_pool(name="pw", bufs=1, space="PSUM") as wpool, \
             tc.tile_pool(name="u", bufs=3) as upool, \
             tc.tile_pool(name="sq", bufs=3) as sqpool, \
             tc.tile_pool(name="s0", bufs=3) as s0pool, \
             tc.tile_pool(name="ob", bufs=3) as obpool:

            # --- constants + phase-1 inputs first: these small DMAs gate
            # the one-hot pipeline, so they go ahead of the big E stream ---
            # iota over (a, k2): value = a  (exact in bf16 for 0..15)
            iotad = cpool.tile([128, 2 * ATILE], bf)
            nc.gpsimd.iota(_ap(iotad[:], [[2, ATILE], [1, 2]]),
                           [[1, ATILE], [0, 2]],
                           channel_multiplier=0,
                           allow_small_or_imprecise_dtypes=True)
            ii_sb = cpool.tile([128, 2 * CH], bf)
            nc.sync.dma_start(out=ii_sb[:], in_=iid2[:])
            fT_sb = mpool.tile([128, CH], fp)
            nc.sync.dma_start(out=fT_sb[:], in_=fT[:])
            r0_sb = mpool.tile([128, CH], bf)
            nc.sync.dma_start(out=r0_sb[:], in_=r0T[:])
            r1_sb = mpool.tile([128, CH], bf)
            nc.sync.dma_start(out=r1_sb[:], in_=r1T[:])
            r2_sb = mpool.tile([128, CH], bf)
            nc.sync.dma_start(out=r2_sb[:], in_=r2T[:])
            e_tiles = {}

            def prefetch(bp):
                # one 1MB DMA per super-pair: 512KB transfers run at ~75%
                # of the 1MB-class DMA rate, so batching both supers buys
                # ~15% effective input bandwidth
                e = epool.tile([128, 2 * EW], bf, tag="esup")
                nc.sync.dma_start(out=e[:],
                                  in_=egd[:, bp * 2 * EW:(bp + 1) * 2 * EW])
                e_tiles[bp] = e

            prefetch(0)  # pair 0's E rides right behind the gating inputs

            wT_sb = cpool.tile([F, F], fp)
            nc.sync.dma_start(out=wT_sb[:], in_=wTd[:])
            wT_bf = cpool.tile([F, F], bf)
            nc.scalar.copy(wT_bf[:], wT_sb[:])
            brow_sb = cpool.tile([1, F], fp)
            nc.sync.dma_start(out=brow_sb[:], in_=browd[:])
            brow_bf = cpool.tile([1, F], bf)
            nc.scalar.copy(brow_bf[:], brow_sb[:])
            c3sb = cpool.tile([1, n_sp * BW2], bf)
            nc.sync.dma_start(out=c3sb[:], in_=c3d[:])

            # rest of the E prefetch window
            for bp in range(1, min(PF // 2, n_sp)):
                prefetch(bp)

            # --- Phase 1: f (bf16), |r|, 1/|r|, f/|r|, Cu4i planes ---
            # produced in a small head span (first 4 pairs) + the
            # remainder, so pair 0's build is gated by ~3us of DVE work
            # instead of the full-width chain (incl the 4.4us reciprocal)
            Cu4i = mpool.tile([128, KBLK * CH], bf)
            fb = mpool.tile([128, CH], bf)
            tA = mpool.tile([128, CH], bf)
            tB = mpool.tile([128, CH], bf)

            def coeff_span(ch0, ch1):
                s_ = slice(ch0, ch1)
                nc.vector.tensor_copy(fb[:, s_], fT_sb[:, s_])
                nc.vector.tensor_tensor(out=tA[:, s_], in0=r0_sb[:, s_],
                                        in1=r0_sb[:, s_], op=mult)
                nc.vector.tensor_tensor(out=tB[:, s_], in0=r1_sb[:, s_],
                                        in1=r1_sb[:, s_], op=mult)
                nc.vector.tensor_tensor(out=tA[:, s_], in0=tA[:, s_],
                                        in1=tB[:, s_], op=add)
                nc.vector.tensor_tensor(out=tB[:, s_], in0=r2_sb[:, s_],
                                        in1=r2_sb[:, s_], op=mult)
                nc.vector.tensor_tensor(out=tA[:, s_], in0=tA[:, s_],
                                        in1=tB[:, s_], op=add)
                nc.scalar.sqrt(tA[:, s_], tA[:, s_])  # |r|
                with nc.allow_low_precision(reason="1/|r| feeds bf16 one-hots"):
                    nc.vector.reciprocal(tB[:, s_], tA[:, s_])
                nc.vector.tensor_tensor(out=tB[:, s_], in0=fb[:, s_],
                                        in1=tB[:, s_], op=mult)

            def cu_span(ch0, ch1):
                w = ch1 - ch0
                ksl = [[KBLK, w]]
                o4 = ch0 * KBLK
                nc.vector.tensor_copy(_ap(Cu4i[:], ksl, off=o4),
                                      fb[:, ch0:ch1])
                nc.vector.tensor_tensor(
                    out=_ap(Cu4i[:], ksl, off=o4 + 1),
                    in0=tB[:, ch0:ch1], in1=r0_sb[:, ch0:ch1], op=mult)
                nc.vector.tensor_tensor(
                    out=_ap(Cu4i[:], ksl, off=o4 + 2),
                    in0=tB[:, ch0:ch1], in1=r1_sb[:, ch0:ch1], op=mult)
                nc.vector.tensor_tensor(
                    out=_ap(Cu4i[:], ksl, off=o4 + 3),
                    in0=tB[:, ch0:ch1], in1=r2_sb[:, ch0:ch1], op=mult)

            HEAD = min(4 * C2, CH)
            coeff_span(0, HEAD)
            cu_span(0, HEAD)

            def build(b):
                """One-hot (cmp + 2 mults) for super-pair b."""
                ch0 = b * C2
                o4 = ch0 * KBLK
                # cmp2[p, (ch, a, k2)] = (iota[a] == ii[ch]), x2-duplicated
                cmp = cmpool.tile([128, C2 * ATILE * 2], bf, tag="cmp")
                nc.vector.tensor_tensor(
                    out=_ap(cmp[:], [[2 * ATILE, C2], [2, ATILE], [1, 2]]),
                    in0=_ap(iotad[:], [[0, C2], [2, ATILE], [1, 2]]),
                    in1=_ap(ii_sb[:], [[2, C2], [0, ATILE], [1, 2]],
                            off=ch0 * 2),
                    op=iseq,
                )
                # O~[p, (ch, a, k)] = cmp2 * Cu4i[ch, k]  (two k-pair passes)
                oh = opool.tile([128, C2 * TW], bf, tag="oh")
                for h in range(2):
                    nc.vector.tensor_tensor(
                        out=_ap(oh[:], [[TW, C2], [KBLK, ATILE], [1, 2]],
                                off=2 * h),
                        in0=_ap(cmp[:], [[2 * ATILE, C2], [2, ATILE], [1, 2]]),
                        in1=_ap(Cu4i[:], [[KBLK, C2], [0, ATILE], [1, 2]],
                                off=o4 + 2 * h),
                        op=mult,
                    )
                return oh

            oh_tiles = {0: build(0)}
            if n_sp > 1:
                oh_tiles[1] = build(1)

            pend = {}   # b -> (U, ob) awaiting stage-1 finish
            pend2 = {}  # b -> (ob, s0t) awaiting stage-2 finish

            def finish1(b):
                """W transform + squares + norm-adds for pair b (one pair
                late: its PE work rides right behind the chunk-MM stream,
                its scalar work fills the W-matmul window). Ua keeps the
                PSUM (tile, atom, k) order verbatim; the W rhs reads the
                u-planes as runs-of-3 (k=1..3 of each atom)."""
                Ua, ob = pend.pop(b)
                pw = wpool.tile([128, 1024], fp, tag="pw")
                for h in range(2):
                    nc.tensor.matmul(
                        out=pw[:, h * 512:h * 512 + 384], lhsT=wT_bf[:],
                        rhs=_ap(Ua[:], [[TW, 8], [KBLK, ATILE], [1, 3]],
                                off=h * 512 + 1),
                        start=True, stop=False)
                    nc.tensor.matmul(
                        out=pw[:, h * 512:h * 512 + 384], lhsT=brow_bf[:1, :],
                        rhs=c3sb[:1, b * BW2 + h * 384:b * BW2 + (h + 1) * 384],
                        start=False, stop=True)
                sq = sqpool.tile([128, BW2], bf, tag="sq")
                nc.scalar.activation(
                    _ap(sq[:], [[384, 2], [1, 384]]),
                    _ap(pw[:], [[512, 2], [1, 384]]),
                    mybir.ActivationFunctionType.Square)
                # sq cols are (t, a, k1..3): sum the 3 planes per (t, a)
                s0t = s0pool.tile([128, 256], bf, tag="s0")
                nc.gpsimd.tensor_tensor(
                    out=_ap(s0t[:], [[ATILE, 16], [1, ATILE]]),
                    in0=_ap(sq[:], [[3 * ATILE, 16], [3, ATILE]]),
                    in1=_ap(sq[:], [[3 * ATILE, 16], [3, ATILE]], off=1),
                    op=add,
                )
                nc.gpsimd.tensor_tensor(
                    out=_ap(s0t[:], [[ATILE, 16], [1, ATILE]]),
                    in0=_ap(s0t[:], [[ATILE, 16], [1, ATILE]]),
                    in1=_ap(sq[:], [[3 * ATILE, 16], [3, ATILE]], off=2),
                    op=add,
                )
                pend2[b] = (ob, s0t)

            def finish2(b):
                """sqrt + store for pair b (two pairs late so the sqrt's
                wait on the gpsimd adds never blocks the scalar FIFO)."""
                ob, s0t = pend2.pop(b)
                nc.scalar.sqrt(ob[:, 0:256], s0t[:])
                # store on the scalar HWDGE ring so its wait never
                # head-of-line-blocks the sync ring's E stream
                nc.scalar.dma_start(out=outd[:, b * 512:(b + 1) * 512],
                                    in_=ob[:])

            for b in range(n_sp):
                if b + PF // 2 < n_sp:
                    prefetch(b + PF // 2)
                oh = oh_tiles.pop(b)
                e = e_tiles.pop(b)

                # segment-sum matmuls: 16 tiles into one 2-bank PSUM tile
                acc2 = ppool.tile([128, 2 * TPS * TW], fp, tag="acc")
                for s2 in range(2):
                    for ti in range(TPS):
                        for ci in range(CPT):
                            cp = s2 * SUP_C + ti * CPT + ci
                            nc.tensor.matmul(
                                out=acc2[:, s2 * 512 + ti * TW:
                                         s2 * 512 + (ti + 1) * TW],
                                lhsT=_ap(e[:], [[1, F]], off=cp * F),
                                rhs=_ap(oh[:], [[1, TW]], off=cp * TW),
                                start=(ci == 0),
                                stop=(ci == CPT - 1),
                            )

                # build pair b+2 before pair b's tail so the DVE FIFO never
                # gates pair b+1/b+2's matmuls behind tail dependencies
                if b + 2 < n_sp:
                    oh_tiles[b + 2] = build(b + 2)
                if b == 0 and HEAD < CH:
                    coeff_span(HEAD, CH)
                    cu_span(HEAD, CH)

                # finish1 BEFORE the U/R drain: the W-matmuls' PSUM-write
                # guard keys on scalar PSUM-reads emitted before them, so
                # emitting U_b/R_b first would chain pair b-1's W-MMs to
                # pair b's U copy (measured 1.9us/pair stall)
                if b > 0:
                    finish1(b - 1)

                # drain acc2 with ONE contiguous verbatim copy (fast
                # 2x-accel scalar path, frees the PSUM buffer quickly);
                # radial plane extracted on the vector engine
                Ua = upool.tile([128, 1024], bf, tag="u")
                nc.scalar.copy(Ua[:], acc2[:])
                ob = obpool.tile([128, 512], bf, tag="ob")
                nc.vector.tensor_copy(
                    _ap(ob[:], [[ATILE, 16], [1, ATILE]], off=256),
                    _ap(Ua[:], [[TW, 16], [KBLK, ATILE]]))
                pend[b] = (Ua, ob)

                if b > 1:
                    finish2(b - 2)
            finish1(n_sp - 1)
            finish2(n_sp - 2)
            finish2(n_sp - 1)

    nc.compile()
    return nc


def host_prep(inputs, n_cores=8):
    """Route pairs to atom-owning cores; variable-base 16-atom pair tiles."""
    emb = np.ascontiguousarray(np.asarray(inputs["atomic_embedding"],
                                          dtype=np.float32))
    # ship the high 16 bits of each fp32 (== the bf16 bit pattern, truncated):
    # a pure byte-slice of the input, no host arithmetic
    embh = np.ascontiguousarray(
        emb.view(np.uint16).reshape(emb.shape[0], -1)[:, 1::2]
    ).view(ml_dtypes.bfloat16)
    f = np.asarray(inputs["f_ij_cutoff"], dtype=np.float32).ravel()
    r = np.asarray(inputs["r_ij"], dtype=np.float32)
    W = np.asarray(inputs["W"], dtype=np.float32)
    b = np.asarray(inputs["b"], dtype=np.float32)
    pl = np.asarray(inputs["pairlist"]).astype(np.int64)
    idx_i, idx_j = pl[0], pl[1]

    N = emb.shape[0]
    P = idx_i.shape[0]
    APC = N // n_cores
    SLOTS = CPT * CHUNK  # pair slots per tile (256)

    cnt_atom = np.bincount(idx_i, minlength=N).astype(np.int64)

    # greedy variable-base tiling per core
    tiles = []  # per core: list of (astart, aend)
    for c in range(n_cores):
        ca = cnt_atom[c * APC:(c + 1) * APC]
        tl = []
        cur, cur_p = 0, 0
        for a in range(APC):
            cp = int(ca[a])
            if cur_p + cp > SLOTS or a - cur >= ATILE:
                tl.append((cur, a))
                cur, cur_p = a, 0
            cur_p += cp
        tl.append((cur, APC))
        tiles.append(tl)
    T = max(len(tl) for tl in tiles)
    T = ((T + 15) // 16) * 16  # multiple of 16 for super-pair tails

    # tile id and base per atom
    tile_of_atom = np.zeros(N, dtype=np.int64)
    base_of_atom = np.zeros(N, dtype=np.int64)
    for c in range(n_cores):
        for t, (a0, a1) in enumerate(tiles[c]):
            tile_of_atom[c * APC + a0:c * APC + a1] = t
            base_of_atom[c * APC + a0:c * APC + a1] = a0

    order = np.argsort(idx_i, kind="stable")
    so_i = idx_i[order]
    core_of = so_i // APC
    key = core_of * T + tile_of_atom[so_i]
    cnt = np.bincount(key, minlength=n_cores * T)
    assert cnt.max() <= SLOTS, cnt.max()
    starts = np.zeros(n_cores * T + 1, dtype=np.int64)
    np.cumsum(cnt, out=starts[1:])
    pos = np.arange(P, dtype=np.int64) - starts[key]
    slot = key * SLOTS + pos
    TOT = n_cores * T * SLOTS

    jj = np.zeros(TOT, dtype=np.int32)  # pad slots: row 0 (one-hot kills it)
    ff = np.zeros(TOT, dtype=np.float32)
    rr = np.zeros((TOT, 3), dtype=np.float32)
    rr[:, 0] = 1.0
    ii = np.full(TOT, 255, dtype=np.int32)  # pad slots: no atom slot
    jj[slot] = idx_j[order]
    ff[slot] = f[order]
    rr[slot] = r[order]
    ii[slot] = so_i - core_of * APC - base_of_atom[so_i]

    CH = T * CPT
    n_sp = T // (2 * TPS)
    BW2 = 2 * TPS * 3 * ATILE
    TOTC = T * SLOTS
    in_maps = []
    out_sel = []  # per core: (valid slot rows, global atom rows)
    for c in range(n_cores):
        sl = slice(c * TOTC, (c + 1) * TOTC)
        tr = lambda x: np.ascontiguousarray(x.reshape(CH, CHUNK).T)
        # host-side gather of neighbor embedding rows, pair-slot order,
        # laid out [pair-in-chunk, (chunk, f)]
        jj_c = jj[sl].reshape(CH, CHUNK)
        eg = np.ascontiguousarray(
            embh[jj_c].transpose(1, 0, 2).reshape(CHUNK, CH * F))
        # slot indices, x2-duplicated: [p, (ch, j2)]
        ii_c = tr(ii[sl].astype(np.float32))  # [128, CH]
        ii2 = np.ascontiguousarray(
            np.repeat(ii_c, 2, axis=1)).astype(ml_dtypes.bfloat16)
        # counts per (super-pair, tile, k-plane, atom)
        cnt3 = np.zeros((T, 3, ATILE), dtype=np.float32)
        rows_slot = []
        rows_atom = []
        for t, (a0, a1) in enumerate(tiles[c]):
            span = a1 - a0
            cnt3[t, :, :span] = cnt_atom[c * APC + a0:c * APC + a1][None, :]
            rows_slot.append(np.arange(t * ATILE, t * ATILE + span))
            rows_atom.append(np.arange(c * APC + a0, c * APC + a1))
        out_sel.append((np.concatenate(rows_slot), np.concatenate(rows_atom)))
        in_maps.append({
            "egd": eg,
            "iid2": ii2,
            "fT": tr(ff[sl]),
            "r0T": tr(rr[sl][:, 0]).astype(ml_dtypes.bfloat16),
            "r1T": tr(rr[sl][:, 1]).astype(ml_dtypes.bfloat16),
            "r2T": tr(rr[sl][:, 2]).astype(ml_dtypes.bfloat16),
            "c3d": np.ascontiguousarray(
                cnt3.reshape(n_sp, 16, 3, ATILE).transpose(0, 1, 3, 2)
                .reshape(1, -1)).astype(ml_dtypes.bfloat16),
            "wTd": np.ascontiguousarray(W.T),
            "browd": np.ascontiguousarray(b.reshape(1, F)),
        })
    return in_maps, dict(N=N, APC=APC, T=T, P=P, out_sel=out_sel)


_NC_CACHE = {}


def kernel(**inputs) -> np.ndarray:
    n_cores = 8
    in_maps, meta = host_prep(inputs, n_cores)
    N = meta["N"]
    T = meta["T"]
    ckey = (N, T, n_cores)
    nc = _NC_CACHE.get(ckey)
    if nc is None:
        nc = build_nc(N, T, n_cores)
        _NC_CACHE[ckey] = nc
    res = run_bass_kernel_spmd(nc, in_maps, core_ids=list(range(n_cores)))
    n_sp = T // (2 * TPS)
    out = np.empty((N, 2 * F), dtype=np.float32)
    for c in range(n_cores):
        # outd [128f, (b, {V,R}, t, a)] -> slot-major rows [T*ATILE, f]
        arr = np.asarray(res.results[c]["outd"]).astype(np.float32)
        v = arr.reshape(128, n_sp, 2, 16, ATILE)
        V = v[:, :, 0].reshape(128, T * ATILE).T
        R = v[:, :, 1].reshape(128, T * ATILE).T
        rows_slot, rows_atom = meta["out_sel"][c]
        out[rows_atom, 0:F] = V[rows_slot]
        out[rows_atom, F:2 * F] = R[rows_slot]
    return out
